# revision 64
# baseline (speedup 1.0000x reference)
"""Multi-head causal attention Bass/Tile kernel for TRN2.

Per-core program (SPMD across 8 cores): each core handles one batch b and
half the heads (HPC=8). Inputs arrive pre-transposed/sliced from the host
in bf16:
  xqT, xkT, xvT : [D, S]   (activations, transposed, bf16)
  wq, wk, wv    : [D, HPC*DK] bf16 (per-core head slice, head-major columns)
  bqp, bkp      : [2*DK, HPC//2]  (bias per head-pair column)
  wo            : [HPC*DK, DO] f32 (slice of Wo rows for these heads)
Output: out [S, DO] = normalized-attention context @ wo  (no bo; host adds
bo + bv@Wo and sums the two head-half partials).

Schedule: chunk-interleaved pipeline. The sequence is processed in 4 column
chunks of 512; for chunk sc we project Q/K/V columns [sc*512,(sc+1)*512) and
then immediately run attention for query block qb=sc (which only needs
K/V/Q chunks <= sc). The exp (scalar engine) work of attention block qb
overlaps the projection matmuls of chunk qb+1 that sit behind it in the PE
queue. The finished context block's output projection is interleaved into
the next attention block's stream.

Causal structure is exploited at 128-column granularity: for the diagonal
key tile g the score matmul and exp only cover query columns >= g*128, and
the mask multiply touches a single 128x128 tile.

Softmax denominators come from a ones-column augmented V (65th row of the
AV matmul output); reciprocals are computed by DMA-repacking the denominator
row across partitions (DVE reciprocal is free-dim serial).
"""

from contextlib import ExitStack

import numpy as np

import concourse.bass as bass
import concourse.mybir as mybir
import concourse.tile as tile
from concourse import library_config

F32 = mybir.dt.float32
F32R = mybir.dt.float32r
BF16 = mybir.dt.bfloat16
AF = mybir.ActivationFunctionType


def split_multiwaits(nc):
    """This walrus build accepts at most one sync-wait per instruction;
    hoist extra waits onto NOPs placed just before the instruction."""
    n_split = 0
    for fn in nc.m.functions:
        for blk in fn.blocks:
            insts = list(blk.instructions)
            out = []
            for inst in insts:
                si = inst.sync_info
                if si is not None and si.on_wait is not None and len(si.on_wait) > 1:
                    waits = list(si.on_wait)
                    for j, w in enumerate(waits[:-1]):
                        nop = mybir.InstNoOp(name=f"{inst.name}-sw{j}", ins=[], outs=[])
                        nop.engine = inst.engine
                        nop.sync_info = mybir.SyncInfo(on_wait=[w], on_update=[])
                        out.append(nop)
                    inst.sync_info = mybir.SyncInfo(
                        on_wait=[waits[-1]], on_update=list(si.on_update or [])
                    )
                    n_split += 1
                out.append(inst)
            if len(out) != len(insts):
                blk.instructions.clear()
                blk.instructions.extend(out)
    return n_split


def build(S=2048, D=1024, HPC=8, DK=64, DO=1024, QB=512, scale=0.125,
          split=True, fuse_mask=True):
    """Build the per-core Bass module. Returns nc."""
    assert S % QB == 0 and D % 128 == 0 and QB % 128 == 0
    n_dt = D // 128          # D tiles (contraction)
    n_sc = S // QB           # proj col chunks == q blocks
    n_qb = S // QB
    n_st = S // 128          # sequence tiles of 128 (key tiles)
    n_kq = QB // 128         # key tiles per q block
    n_pairs = HPC // 2
    HD = HPC * DK            # local head-concat dim
    n_ht = HD // 128         # ctx_stack tiles
    VA = DK + 1              # V augmented with ones column
    FQ = QB // 128
    NCK = 512                # outproj column chunk (f32r moving max 512)

    nc = bass.Bass("TRN2", target_bir_lowering=False, debug=False)

    # activations/weights arrive pre-tiled from the host so each DMA is one
    # contiguous run per partition: x [n_sc, 128, n_dt*QB], w [128, n_dt*HD]
    xqT = nc.dram_tensor("xqT", [n_sc, 128, n_dt * QB], BF16,
                         kind="ExternalInput").ap()
    xkT = nc.dram_tensor("xkT", [n_sc, 128, n_dt * QB], BF16,
                         kind="ExternalInput").ap()
    xvT = nc.dram_tensor("xvT", [n_sc, 128, n_dt * QB], BF16,
                         kind="ExternalInput").ap()
    wq = nc.dram_tensor("wq", [128, n_dt * HD], BF16, kind="ExternalInput").ap()
    wk = nc.dram_tensor("wk", [128, n_dt * HD], BF16, kind="ExternalInput").ap()
    wv = nc.dram_tensor("wv", [128, n_dt * HD], BF16, kind="ExternalInput").ap()
    bqp = nc.dram_tensor("bqp", [2 * DK, n_pairs], F32, kind="ExternalInput").ap()
    bkp = nc.dram_tensor("bkp", [2 * DK, n_pairs], F32, kind="ExternalInput").ap()
    wo = nc.dram_tensor("wo", [128, n_ht * DO], F32R, kind="ExternalInput").ap()
    mask_in = nc.dram_tensor("mask_in", [128, 128], BF16, kind="ExternalInput").ap()
    maskm_in = nc.dram_tensor("maskm_in", [128, 128], BF16,
                              kind="ExternalInput").ap()
    ident_in = nc.dram_tensor("ident_in", [128, 128], BF16,
                              kind="ExternalInput").ap()
    vones = nc.dram_tensor("vones", [128, HPC], BF16, kind="ExternalInput").ap()
    out = nc.dram_tensor("out", [S, DO], F32, kind="ExternalOutput").ap()

    with tile.TileContext(nc) as tc:
        ctx = ExitStack()
        # ---- persistent pools ----
        qk_pool = ctx.enter_context(tc.tile_pool(name="qk", bufs=1))
        va_pool = ctx.enter_context(tc.tile_pool(name="va", bufs=1))
        cs_pool = ctx.enter_context(tc.tile_pool(name="cs", bufs=1))
        w_pool = ctx.enter_context(tc.tile_pool(name="w", bufs=1))
        x_pool = ctx.enter_context(tc.tile_pool(name="x", bufs=1))
        small_pool = ctx.enter_context(tc.tile_pool(name="small", bufs=1))
        expool = ctx.enter_context(tc.tile_pool(name="ex", bufs=4))
        rcpool = ctx.enter_context(tc.tile_pool(name="rc", bufs=4))
        dscrpool = ctx.enter_context(tc.tile_pool(name="dscr", bufs=8, space="DRAM"))
        oev = ctx.enter_context(tc.tile_pool(name="oev", bufs=4))
        # PSUM: sc 2 bufs x 2 banks + ctx 2 banks + shared proj/outproj 2 banks
        psc = ctx.enter_context(tc.tile_pool(name="psc", bufs=2, space="PSUM"))
        pctx = ctx.enter_context(tc.tile_pool(name="pctx", bufs=1, space="PSUM"))
        pmix = ctx.enter_context(tc.tile_pool(name="pmix", bufs=2, space="PSUM"))

        qt_sb = [qk_pool.tile([2 * DK, S], BF16, name=f"qt{p}", tag=f"qt{p}")
                 for p in range(n_pairs)]
        kt_sb = [qk_pool.tile([2 * DK, S], BF16, name=f"kt{p}", tag=f"kt{p}")
                 for p in range(n_pairs)]
        v_aug = [va_pool.tile([128, HPC * VA], BF16, name=f"va{t}", tag=f"va{t}")
                 for t in range(n_st)]
        ctx_stack = [cs_pool.tile([128, S], F32R, name=f"cs{t}", tag=f"cs{t}")
                     for t in range(n_ht)]

        # ---- weights: one big DMA each (d-major free dim) ----
        wq_sb = w_pool.tile([128, n_dt * HD], BF16, tag="wq")
        wk_sb = w_pool.tile([128, n_dt * HD], BF16, tag="wk")
        wv_sb = w_pool.tile([128, n_dt * HD], BF16, tag="wv")
        wo_sb = w_pool.tile([128, n_ht * DO], F32R, tag="wo")
        x_sb = {}
        for t in ("q", "k", "v"):
            for sc in range(n_sc):
                x_sb[(t, sc)] = x_pool.tile([128, n_dt * QB], BF16,
                                            name=f"x{t}{sc}", tag=f"x{t}", bufs=2)

        bq_sb = small_pool.tile([2 * DK, n_pairs], F32, tag="bq")
        bk_sb = small_pool.tile([2 * DK, n_pairs], F32, tag="bk")
        mask_sb = small_pool.tile([128, 128], BF16, tag="mask")
        maskm_sb = small_pool.tile([128, 128], BF16, tag="maskm")
        ident_sb = small_pool.tile([128, 128], BF16, tag="ident")
        vones_sb = small_pool.tile([128, HPC], BF16, tag="vones")

        # critical-path-first DMA order, split across the two HWDGE queues
        # (sync + scalar) so the first Q-proj matmul unblocks in ~3us.
        def dma_xt(eng, t, xT, sc):
            eng.dma_start(x_sb[(t, sc)][:], xT[sc])

        def dma_x(sc):
            for (t, xT) in (("q", xqT), ("k", xkT), ("v", xvT)):
                dma_xt(nc.sync, t, xT, sc)

        nc.scalar.dma_start(wq_sb[:], wq[:])
        dma_xt(nc.sync, "q", xqT, 0)
        nc.sync.dma_start(wk_sb[:], wk[:])
        dma_xt(nc.scalar, "k", xkT, 0)
        nc.scalar.dma_start(wv_sb[:], wv[:])
        dma_xt(nc.sync, "v", xvT, 0)
        nc.sync.dma_start(bq_sb[:], bqp[:])
        nc.sync.dma_start(bk_sb[:], bkp[:])
        nc.sync.dma_start(mask_sb[:], mask_in[:])
        nc.sync.dma_start(maskm_sb[:], maskm_in[:])
        nc.sync.dma_start(ident_sb[:], ident_in[:])
        nc.sync.dma_start(vones_sb[:], vones[:])
        nc.scalar.dma_start(wo_sb[:], wo[:])
        if n_sc > 1:
            dma_xt(nc.sync, "q", xqT, 1)
            dma_xt(nc.scalar, "k", xkT, 1)
            dma_xt(nc.sync, "v", xvT, 1)

        def proj_thunks(sc):
            # Q and K projections for columns [sc*QB, (sc+1)*QB), as a list
            # of closures (one per pair / v-subtile; ~3.4us PE each) so they
            # can be interleaved into the previous attention block's stream.
            thunks = []
            for (w_all, b_sb, dst, t) in ((wq_sb, bq_sb, qt_sb, "q"),
                                          (wk_sb, bk_sb, kt_sb, "k")):
                xt = x_sb[(t, sc)]
                for p in range(n_pairs):
                    def th(w_all=w_all, b_sb=b_sb, dst=dst, t=t, xt=xt, p=p):
                        ps = pmix.tile([128, QB], F32, name=f"pj{t}{sc}{p}",
                                       tag="mix")
                        for d in range(n_dt):
                            nc.tensor.matmul(
                                ps[:],
                                w_all[:, d * HD + p * 128:d * HD + (p + 1) * 128],
                                xt[:, d * QB:(d + 1) * QB],
                                start=(d == 0), stop=(d == n_dt - 1))
                        nc.vector.tensor_scalar_add(
                            dst[p][:, sc * QB:(sc + 1) * QB], ps[:],
                            b_sb[:, p:p + 1])
                    thunks.append(th)
            xt = x_sb[("v", sc)]
            for stl in range(n_kq):
                def th(xt=xt, stl=stl):
                    st = sc * n_kq + stl
                    ps = pmix.tile([128, HD], F32, name=f"pjv{sc}{stl}", tag="mix")
                    for d in range(n_dt):
                        nc.tensor.matmul(
                            ps[:], xt[:, d * QB + stl * 128:d * QB + (stl + 1) * 128],
                            wv_sb[:, d * HD:(d + 1) * HD],
                            start=(d == 0), stop=(d == n_dt - 1))
                    va3 = v_aug[st][:].rearrange("p (h c) -> p h c", c=VA)
                    nc.scalar.copy(va3[:, :, 0:DK],
                                   ps[:].rearrange("p (h c) -> p h c", c=DK))
                    nc.vector.tensor_copy(va3[:, :, DK:VA],
                                          vones_sb[:].rearrange("p h -> p h ()"))
                thunks.append(th)
            return thunks

        def evac_and_normalize(h, qb, ctx_ps):
            # evacuate ctx (unnormalized) into ctx_stack + denom row, then
            # normalize in place once the PE-free recip chain lands. All hops
            # ride the sync HWDGE queue: triggers on the scalar queue would
            # block the ACT FIFO behind their data dependencies and stall exps.
            eng = nc.sync
            t, row0 = h // 2, (h % 2) * DK
            cs_slice = ctx_stack[t][row0:row0 + DK, qb * QB:(qb + 1) * QB]
            dn = rcpool.tile([1, QB], F32, name=f"dn{h}_{qb}", tag="dn")
            nc.vector.tensor_copy(dn[:], ctx_ps[DK:DK + 1, :])
            nc.vector.tensor_copy(cs_slice, ctx_ps[0:DK, :])
            ds1 = dscrpool.tile([1, QB], F32, name=f"ds1_{h}_{qb}", tag="ds1")
            eng.dma_start(ds1[:], dn[:])
            dnp = rcpool.tile([128, FQ], F32, name=f"dnp{h}_{qb}", tag="dnp")
            eng.dma_start(dnp[:], ds1[0, :].rearrange("(p f) -> p f", f=FQ))
            rcp = rcpool.tile([128, FQ], F32R, name=f"rcp{h}_{qb}", tag="rcp")
            with nc.allow_low_precision(reason="denom recip"):
                nc.vector.reciprocal(rcp[:], dnp[:])
            ds2 = dscrpool.tile([1, QB], F32R, name=f"ds2_{h}_{qb}", tag="ds2")
            eng.dma_start(ds2[0, :].rearrange("(p f) -> p f", f=FQ), rcp[:])
            bc_sb = rcpool.tile([128, QB], F32R, name=f"bc{h}_{qb}", tag="bc")
            eng.dma_start(bc_sb[row0:row0 + DK, :],
                          ds2[:].broadcast_to([DK, QB]))
            nc.vector.tensor_mul(cs_slice, cs_slice, bc_sb[row0:row0 + DK, :])

        def outproj_thunks(qb):
            # project finished ctx_stack columns (q rows qb*QB..) through wo
            thunks = []
            for stl in range(n_kq):
                st = qb * n_kq + stl
                for nck in range(DO // NCK):
                    def th(st=st, stl=stl, nck=nck):
                        ps = pmix.tile([128, NCK], F32, name=f"po{qb}{stl}{nck}",
                                       tag="mix")
                        for t in range(n_ht):
                            nc.tensor.matmul(
                                ps[:], ctx_stack[t][:, st * 128:(st + 1) * 128],
                                wo_sb[:, t * DO + nck * NCK:t * DO + (nck + 1) * NCK],
                                start=(t == 0), stop=(t == n_ht - 1))
                        ev = oev.tile([128, NCK], F32, name=f"oe{qb}{stl}{nck}",
                                      tag="ev")
                        if nck % 2 == 0:
                            nc.vector.tensor_copy(ev[:], ps[:])
                        else:
                            nc.scalar.copy(ev[:], ps[:])
                        nc.sync.dma_start(out[st * 128:(st + 1) * 128,
                                              nck * NCK:(nck + 1) * NCK], ev[:])
                    thunks.append(th)
            return thunks

        def att_block(qb, bg):
            # attention for query block qb; interleave background thunks
            # (next proj chunk + previous outproj) into the kt stream.
            ktm = (qb + 1) * n_kq - 1
            n_iters = n_pairs * (ktm + 1)
            it = 0
            done = 0
            for p in range(n_pairs):
                he, ho = 2 * p, 2 * p + 1
                ctx_e = pctx.tile([VA, QB], F32, name=f"ce{qb}{p}", tag="ctx_e")
                ctx_o = pctx.tile([VA, QB], F32, name=f"co{qb}{p}", tag="ctx_o")
                q0 = qb * QB
                for kt in range(ktm + 1):
                    g = kt - qb * n_kq
                    qc0 = max(0, g) * 128
                    sct = psc.tile([128, 2 * QB], F32, name=f"s{qb}{p}{kt}",
                                   tag="sc")
                    sc3 = sct[:].rearrange("p (h c) -> p h c", c=QB)
                    for hh, r0, r1 in ((0, 0, DK), (1, DK, 2 * DK)):
                        if g >= 0:
                            # diagonal tile: causal mask folded into the score
                            # accumulation as a -1e9 upper-triangular bias
                            # (identity-stationary matmul) so exp yields exact
                            # zeros and no post-exp mask multiply is needed.
                            nc.tensor.matmul(
                                sc3[:, hh, qc0:qc0 + 128],
                                kt_sb[p][r0:r1, kt * 128:(kt + 1) * 128],
                                qt_sb[p][r0:r1, q0 + qc0:q0 + qc0 + 128],
                                start=True, stop=False)
                            nc.tensor.matmul(
                                sc3[:, hh, qc0:qc0 + 128], ident_sb[:],
                                maskm_sb[:], start=False, stop=True)
                            if qc0 + 128 < QB:
                                nc.tensor.matmul(
                                    sc3[:, hh, qc0 + 128:QB],
                                    kt_sb[p][r0:r1, kt * 128:(kt + 1) * 128],
                                    qt_sb[p][r0:r1, q0 + qc0 + 128:q0 + QB],
                                    start=True, stop=True)
                        else:
                            nc.tensor.matmul(
                                sc3[:, hh, 0:QB],
                                kt_sb[p][r0:r1, kt * 128:(kt + 1) * 128],
                                qt_sb[p][r0:r1, q0:q0 + QB],
                                start=True, stop=True)
                    ex = expool.tile([128, 2 * QB], BF16, name=f"e{qb}{p}{kt}",
                                     tag="ex")
                    ex3 = ex[:].rearrange("p (h c) -> p h c", c=QB)
                    nc.scalar.activation(ex3[:, :, qc0:QB], sc3[:, :, qc0:QB],
                                         AF.Exp, scale=scale)
                    # emit background PE work here: it executes while the
                    # scalar engine computes this iteration's exp, hiding the
                    # in-order PE stall at the AV matmul below.
                    it += 1
                    while done * n_iters < it * len(bg):
                        bg[done]()
                        done += 1
                    for (h, hh, cx) in ((he, 0, ctx_e), (ho, 1, ctx_o)):
                        vsl = v_aug[kt][:, h * VA:(h + 1) * VA]
                        if g >= 0:
                            nc.tensor.matmul(
                                cx[:, qc0:qc0 + 128], vsl,
                                ex3[:, hh, qc0:qc0 + 128],
                                start=(kt == 0), stop=True)
                            if qc0 + 128 < QB:
                                nc.tensor.matmul(
                                    cx[:, qc0 + 128:QB], vsl,
                                    ex3[:, hh, qc0 + 128:QB],
                                    start=(kt == 0), stop=False)
                        else:
                            nc.tensor.matmul(
                                cx[:], vsl, ex3[:, hh, 0:QB],
                                start=(kt == 0), stop=False)
                evac_and_normalize(he, qb, ctx_e)
                evac_and_normalize(ho, qb, ctx_o)
            while done < len(bg):
                bg[done]()
                done += 1

        def interleave(a, b):
            # spread b evenly through a
            res = list(a)
            for i, x in enumerate(b):
                res.insert((i + 1) * (len(a) + len(b)) // (len(b) + 1), x)
            return res

        # background-work assignment balances the PE's spare capacity against
        # each attention block's scalar-engine (exp) pacing: the last block
        # gets the two ripest outproj blocks, its projection having been
        # pulled one block earlier.
        for th in proj_thunks(0):
            th()
        for sc in range(n_sc):
            if sc + 2 <= n_sc - 1:
                dma_x(sc + 2)
            bg = proj_thunks(sc + 1) if sc + 1 < n_sc else []
            if sc > 0:
                bg = interleave(bg, outproj_thunks(sc - 1))
            att_block(sc, bg)
        for th in outproj_thunks(n_qb - 1):
            th()
        ctx.close()

    if split:
        split_multiwaits(nc)
    return nc


def core_inputs(queries, keys, values, Wq, bq, Wk, bk, Wv, bv, Wo, core, n_cores=8,
                HPC=None):
    """Host-side shard prep for one core. core -> (batch, head-group)."""
    import ml_dtypes
    bf16 = ml_dtypes.bfloat16
    B = queries.shape[0]
    H = Wq.shape[0]
    groups = n_cores // B
    b, hg = core // groups, core % groups
    if HPC is None:
        HPC = H // groups
    h0 = hg * HPC
    DK = Wq.shape[2]

    D = Wq.shape[1]
    HD = HPC * DK
    S = queries.shape[1]
    QB = 512
    n_dt, n_sc, n_ht = D // 128, S // QB, HD // 128

    def wsel(W):
        # [H, D, dk] -> [D, HPC*dk] head-major -> pre-tiled [128, n_dt*HD]
        w = W[h0:h0 + HPC].transpose(1, 0, 2).reshape(D, HD)
        return np.ascontiguousarray(
            w.reshape(n_dt, 128, HD).transpose(1, 0, 2).reshape(128, n_dt * HD)
        ).astype(bf16)

    def xsel(x):
        # [S, D] -> xT [D, S] -> pre-tiled [n_sc, 128, n_dt*QB]
        xT = x.T.reshape(n_dt, 128, n_sc, QB)
        return np.ascontiguousarray(
            xT.transpose(2, 1, 0, 3).reshape(n_sc, 128, n_dt * QB)).astype(bf16)

    def bpairs(bias):
        # [H, dk] -> [2*dk, HPC//2]
        bsel = bias[h0:h0 + HPC].reshape(HPC // 2, 2 * DK)
        return np.ascontiguousarray(bsel.T)

    wo_sel = Wo[h0 * DK:(h0 + HPC) * DK, :]
    DO = wo_sel.shape[1]
    wo_tiled = np.ascontiguousarray(
        wo_sel.reshape(n_ht, 128, DO).transpose(1, 0, 2).reshape(128, n_ht * DO))

    r = np.arange(128)
    mask = (r[None, :] >= r[:, None]).astype(np.float32)  # col >= row
    return {
        "mask_in": mask.astype(bf16),
        "maskm_in": ((1.0 - mask) * -1e9).astype(bf16),
        "ident_in": np.eye(128, dtype=np.float32).astype(bf16),
        "vones": np.ones((128, HPC), bf16),
        "xqT": xsel(queries[b]),
        "xkT": xsel(keys[b]),
        "xvT": xsel(values[b]),
        "wq": wsel(Wq), "wk": wsel(Wk), "wv": wsel(Wv),
        "bqp": bpairs(bq), "bkp": bpairs(bk),
        "wo": wo_tiled,
    }


def assemble(results, B, n_cores, bias_total):
    """Sum head-group partials per batch and add the host-side bias."""
    groups = n_cores // B
    outs = []
    for b in range(B):
        acc = results[b * groups]["out"].astype(np.float64)
        for g in range(1, groups):
            acc = acc + results[b * groups + g]["out"]
        outs.append(acc + bias_total)
    return np.stack(outs).astype(np.float32)


# ---------------------------------------------------------------------------
# Harness entry point: full (unsharded) inputs -> full output.
# Shards batch (4) x head-halves (2) across the 8 NeuronCores, runs the Bass
# kernel via run_bass_kernel_spmd, then sums head-half partials per batch on
# the host (+ bias fold: out += bo + bv @ Wo, exact because attention rows
# sum to 1 after normalization).
# ---------------------------------------------------------------------------
_CACHE = {}


def kernel(**inputs):
    from concourse.bass_utils import run_bass_kernel_spmd

    queries = np.asarray(inputs["queries"], np.float32)
    keys = np.asarray(inputs["keys"], np.float32)
    values = np.asarray(inputs["values"], np.float32)
    Wq = np.asarray(inputs["Wq"], np.float32)
    bq = np.asarray(inputs["bq"], np.float32)
    Wk = np.asarray(inputs["Wk"], np.float32)
    bk = np.asarray(inputs["bk"], np.float32)
    Wv = np.asarray(inputs["Wv"], np.float32)
    bv = np.asarray(inputs["bv"], np.float32)
    Wo = np.asarray(inputs["Wo"], np.float32)
    bo = np.asarray(inputs["bo"], np.float32)

    B = queries.shape[0]
    n_cores = 8
    if "nc" not in _CACHE:
        _CACHE["nc"] = build()
    nc = _CACHE["nc"]
    in_maps = [core_inputs(queries, keys, values, Wq, bq, Wk, bk, Wv, bv, Wo,
                           core=c, n_cores=n_cores) for c in range(n_cores)]
    res = run_bass_kernel_spmd(nc, in_maps, list(range(n_cores)))
    bias_total = bo + bv.reshape(-1) @ Wo
    return assemble(res.results, B, n_cores, bias_total)


# revision 66
# speedup vs baseline: 1.0192x; 1.0192x over previous
"""Multi-head causal attention Bass/Tile kernel for TRN2.

Per-core program (SPMD across 8 cores): each core handles one batch b and
half the heads (HPC=8). Inputs arrive pre-transposed/sliced from the host
in bf16:
  xqT, xkT, xvT : [D, S]   (activations, transposed, bf16)
  wq, wk, wv    : [D, HPC*DK] bf16 (per-core head slice, head-major columns)
  bqp, bkp      : [2*DK, HPC//2]  (bias per head-pair column)
  wo            : [HPC*DK, DO] f32 (slice of Wo rows for these heads)
Output: out [S, DO] = normalized-attention context @ wo  (no bo; host adds
bo + bv@Wo and sums the two head-half partials).

Schedule: chunk-interleaved pipeline. The sequence is processed in 4 column
chunks of 512; for chunk sc we project Q/K/V columns [sc*512,(sc+1)*512) and
then immediately run attention for query block qb=sc (which only needs
K/V/Q chunks <= sc). The exp (scalar engine) work of attention block qb
overlaps the projection matmuls of chunk qb+1 that sit behind it in the PE
queue. The finished context block's output projection is interleaved into
the next attention block's stream.

Causal structure is exploited at 128-column granularity: for the diagonal
key tile g the score matmul and exp only cover query columns >= g*128, and
the mask multiply touches a single 128x128 tile.

Softmax denominators come from a ones-column augmented V (65th row of the
AV matmul output); reciprocals are computed by DMA-repacking the denominator
row across partitions (DVE reciprocal is free-dim serial).
"""

from contextlib import ExitStack

import numpy as np

import concourse.bass as bass
import concourse.mybir as mybir
import concourse.tile as tile
from concourse import library_config

F32 = mybir.dt.float32
F32R = mybir.dt.float32r
BF16 = mybir.dt.bfloat16
AF = mybir.ActivationFunctionType


def split_multiwaits(nc):
    """This walrus build accepts at most one sync-wait per instruction;
    hoist extra waits onto NOPs placed just before the instruction."""
    n_split = 0
    for fn in nc.m.functions:
        for blk in fn.blocks:
            insts = list(blk.instructions)
            out = []
            for inst in insts:
                si = inst.sync_info
                if si is not None and si.on_wait is not None and len(si.on_wait) > 1:
                    waits = list(si.on_wait)
                    for j, w in enumerate(waits[:-1]):
                        nop = mybir.InstNoOp(name=f"{inst.name}-sw{j}", ins=[], outs=[])
                        nop.engine = inst.engine
                        nop.sync_info = mybir.SyncInfo(on_wait=[w], on_update=[])
                        out.append(nop)
                    inst.sync_info = mybir.SyncInfo(
                        on_wait=[waits[-1]], on_update=list(si.on_update or [])
                    )
                    n_split += 1
                out.append(inst)
            if len(out) != len(insts):
                blk.instructions.clear()
                blk.instructions.extend(out)
    return n_split


def build(S=2048, D=1024, HPC=8, DK=64, DO=1024, QB=512, scale=0.125,
          split=True, fuse_mask=True):
    """Build the per-core Bass module. Returns nc."""
    assert S % QB == 0 and D % 128 == 0 and QB % 128 == 0
    n_dt = D // 128          # D tiles (contraction)
    n_sc = S // QB           # proj col chunks == q blocks
    n_qb = S // QB
    n_st = S // 128          # sequence tiles of 128 (key tiles)
    n_kq = QB // 128         # key tiles per q block
    n_pairs = HPC // 2
    HD = HPC * DK            # local head-concat dim
    n_ht = HD // 128         # ctx_stack tiles
    VA = DK + 1              # V augmented with ones column
    FQ = QB // 128
    NCK = 512                # outproj column chunk (f32r moving max 512)

    nc = bass.Bass("TRN2", target_bir_lowering=False, debug=False)

    # activations/weights arrive pre-tiled from the host so each DMA is one
    # contiguous run per partition: x [n_sc, 128, n_dt*QB], w [128, n_dt*HD]
    xqT = nc.dram_tensor("xqT", [n_sc, 128, n_dt * QB], BF16,
                         kind="ExternalInput").ap()
    xkT = nc.dram_tensor("xkT", [n_sc, 128, n_dt * QB], BF16,
                         kind="ExternalInput").ap()
    xvT = nc.dram_tensor("xvT", [n_sc, 128, n_dt * QB], BF16,
                         kind="ExternalInput").ap()
    wq = nc.dram_tensor("wq", [128, n_dt * HD], BF16, kind="ExternalInput").ap()
    wk = nc.dram_tensor("wk", [128, n_dt * HD], BF16, kind="ExternalInput").ap()
    wv = nc.dram_tensor("wv", [128, n_dt * HD], BF16, kind="ExternalInput").ap()
    bqp = nc.dram_tensor("bqp", [2 * DK, n_pairs], F32, kind="ExternalInput").ap()
    bkp = nc.dram_tensor("bkp", [2 * DK, n_pairs], F32, kind="ExternalInput").ap()
    wo = nc.dram_tensor("wo", [128, n_ht * DO], F32R, kind="ExternalInput").ap()
    mask_in = nc.dram_tensor("mask_in", [128, 128], BF16, kind="ExternalInput").ap()
    vones = nc.dram_tensor("vones", [128, HPC], BF16, kind="ExternalInput").ap()
    out = nc.dram_tensor("out", [S, DO], F32, kind="ExternalOutput").ap()

    with tile.TileContext(nc) as tc:
        ctx = ExitStack()
        # ---- persistent pools ----
        qk_pool = ctx.enter_context(tc.tile_pool(name="qk", bufs=1))
        va_pool = ctx.enter_context(tc.tile_pool(name="va", bufs=1))
        cs_pool = ctx.enter_context(tc.tile_pool(name="cs", bufs=1))
        w_pool = ctx.enter_context(tc.tile_pool(name="w", bufs=1))
        x_pool = ctx.enter_context(tc.tile_pool(name="x", bufs=1))
        small_pool = ctx.enter_context(tc.tile_pool(name="small", bufs=1))
        expool = ctx.enter_context(tc.tile_pool(name="ex", bufs=4))
        rcpool = ctx.enter_context(tc.tile_pool(name="rc", bufs=4))
        dscrpool = ctx.enter_context(tc.tile_pool(name="dscr", bufs=8, space="DRAM"))
        oev = ctx.enter_context(tc.tile_pool(name="oev", bufs=4))
        # PSUM: sc 2 bufs x 2 banks + ctx 2 banks + shared proj/outproj 2 banks
        psc = ctx.enter_context(tc.tile_pool(name="psc", bufs=2, space="PSUM"))
        pctx = ctx.enter_context(tc.tile_pool(name="pctx", bufs=1, space="PSUM"))
        pmix = ctx.enter_context(tc.tile_pool(name="pmix", bufs=2, space="PSUM"))

        qt_sb = [qk_pool.tile([2 * DK, S], BF16, name=f"qt{p}", tag=f"qt{p}")
                 for p in range(n_pairs)]
        kt_sb = [qk_pool.tile([2 * DK, S], BF16, name=f"kt{p}", tag=f"kt{p}")
                 for p in range(n_pairs)]
        v_aug = [va_pool.tile([128, HPC * VA], BF16, name=f"va{t}", tag=f"va{t}")
                 for t in range(n_st)]
        ctx_stack = [cs_pool.tile([128, S], F32R, name=f"cs{t}", tag=f"cs{t}")
                     for t in range(n_ht)]

        # ---- weights: one big DMA each (d-major free dim) ----
        wq_sb = w_pool.tile([128, n_dt * HD], BF16, tag="wq")
        wk_sb = w_pool.tile([128, n_dt * HD], BF16, tag="wk")
        wv_sb = w_pool.tile([128, n_dt * HD], BF16, tag="wv")
        wo_sb = w_pool.tile([128, n_ht * DO], F32R, tag="wo")
        x_sb = {}
        for t in ("q", "k", "v"):
            for sc in range(n_sc):
                x_sb[(t, sc)] = x_pool.tile([128, n_dt * QB], BF16,
                                            name=f"x{t}{sc}", tag=f"x{t}", bufs=2)

        bq_sb = small_pool.tile([2 * DK, n_pairs], F32, tag="bq")
        bk_sb = small_pool.tile([2 * DK, n_pairs], F32, tag="bk")
        mask_sb = small_pool.tile([128, 128], BF16, tag="mask")
        vones_sb = small_pool.tile([128, HPC], BF16, tag="vones")

        # critical-path-first DMA order, split across the two HWDGE queues
        # (sync + scalar) so the first Q-proj matmul unblocks in ~3us.
        def dma_xt(eng, t, xT, sc):
            eng.dma_start(x_sb[(t, sc)][:], xT[sc])

        def dma_x(sc):
            for (t, xT) in (("q", xqT), ("k", xkT), ("v", xvT)):
                dma_xt(nc.sync, t, xT, sc)

        nc.scalar.dma_start(wq_sb[:], wq[:])
        dma_xt(nc.sync, "q", xqT, 0)
        nc.sync.dma_start(wk_sb[:], wk[:])
        dma_xt(nc.scalar, "k", xkT, 0)
        nc.scalar.dma_start(wv_sb[:], wv[:])
        dma_xt(nc.sync, "v", xvT, 0)
        nc.sync.dma_start(bq_sb[:], bqp[:])
        nc.sync.dma_start(bk_sb[:], bkp[:])
        nc.sync.dma_start(mask_sb[:], mask_in[:])
        nc.sync.dma_start(vones_sb[:], vones[:])
        nc.scalar.dma_start(wo_sb[:], wo[:])
        if n_sc > 1:
            dma_xt(nc.sync, "q", xqT, 1)
            dma_xt(nc.scalar, "k", xkT, 1)
            dma_xt(nc.sync, "v", xvT, 1)

        def proj_thunks(sc):
            # Q and K projections for columns [sc*QB, (sc+1)*QB), as a list
            # of closures (one per pair / v-subtile; ~3.4us PE each) so they
            # can be interleaved into the previous attention block's stream.
            thunks = []
            for (w_all, b_sb, dst, t) in ((wq_sb, bq_sb, qt_sb, "q"),
                                          (wk_sb, bk_sb, kt_sb, "k")):
                xt = x_sb[(t, sc)]
                for p in range(n_pairs):
                    def th(w_all=w_all, b_sb=b_sb, dst=dst, t=t, xt=xt, p=p):
                        ps = pmix.tile([128, QB], F32, name=f"pj{t}{sc}{p}",
                                       tag="mix")
                        for d in range(n_dt):
                            nc.tensor.matmul(
                                ps[:],
                                w_all[:, d * HD + p * 128:d * HD + (p + 1) * 128],
                                xt[:, d * QB:(d + 1) * QB],
                                start=(d == 0), stop=(d == n_dt - 1))
                        nc.vector.tensor_scalar_add(
                            dst[p][:, sc * QB:(sc + 1) * QB], ps[:],
                            b_sb[:, p:p + 1])
                    thunks.append(th)
            xt = x_sb[("v", sc)]
            for stl in range(n_kq):
                def th(xt=xt, stl=stl):
                    st = sc * n_kq + stl
                    ps = pmix.tile([128, HD], F32, name=f"pjv{sc}{stl}", tag="mix")
                    for d in range(n_dt):
                        nc.tensor.matmul(
                            ps[:], xt[:, d * QB + stl * 128:d * QB + (stl + 1) * 128],
                            wv_sb[:, d * HD:(d + 1) * HD],
                            start=(d == 0), stop=(d == n_dt - 1))
                    va3 = v_aug[st][:].rearrange("p (h c) -> p h c", c=VA)
                    nc.scalar.copy(va3[:, :, 0:DK],
                                   ps[:].rearrange("p (h c) -> p h c", c=DK))
                    nc.vector.tensor_copy(va3[:, :, DK:VA],
                                          vones_sb[:].rearrange("p h -> p h ()"))
                thunks.append(th)
            return thunks

        def evac_and_normalize(h, qb, ctx_ps):
            # evacuate ctx (unnormalized) into ctx_stack + denom row, then
            # normalize in place once the PE-free recip chain lands. All hops
            # ride the sync HWDGE queue: triggers on the scalar queue would
            # block the ACT FIFO behind their data dependencies and stall exps.
            eng = nc.sync
            t, row0 = h // 2, (h % 2) * DK
            cs_slice = ctx_stack[t][row0:row0 + DK, qb * QB:(qb + 1) * QB]
            dn = rcpool.tile([1, QB], F32, name=f"dn{h}_{qb}", tag="dn")
            nc.vector.tensor_copy(dn[:], ctx_ps[DK:DK + 1, :])
            nc.vector.tensor_copy(cs_slice, ctx_ps[0:DK, :])
            ds1 = dscrpool.tile([1, QB], F32, name=f"ds1_{h}_{qb}", tag="ds1")
            eng.dma_start(ds1[:], dn[:])
            dnp = rcpool.tile([128, FQ], F32, name=f"dnp{h}_{qb}", tag="dnp")
            eng.dma_start(dnp[:], ds1[0, :].rearrange("(p f) -> p f", f=FQ))
            rcp = rcpool.tile([128, FQ], F32R, name=f"rcp{h}_{qb}", tag="rcp")
            with nc.allow_low_precision(reason="denom recip"):
                nc.vector.reciprocal(rcp[:], dnp[:])
            ds2 = dscrpool.tile([1, QB], F32R, name=f"ds2_{h}_{qb}", tag="ds2")
            eng.dma_start(ds2[0, :].rearrange("(p f) -> p f", f=FQ), rcp[:])
            bc_sb = rcpool.tile([128, QB], F32R, name=f"bc{h}_{qb}", tag="bc")
            eng.dma_start(bc_sb[row0:row0 + DK, :],
                          ds2[:].broadcast_to([DK, QB]))
            nc.vector.tensor_mul(cs_slice, cs_slice, bc_sb[row0:row0 + DK, :])

        def outproj_thunks(qb):
            # project finished ctx_stack columns (q rows qb*QB..) through wo
            thunks = []
            for stl in range(n_kq):
                st = qb * n_kq + stl
                for nck in range(DO // NCK):
                    def th(st=st, stl=stl, nck=nck):
                        ps = pmix.tile([128, NCK], F32, name=f"po{qb}{stl}{nck}",
                                       tag="mix")
                        for t in range(n_ht):
                            nc.tensor.matmul(
                                ps[:], ctx_stack[t][:, st * 128:(st + 1) * 128],
                                wo_sb[:, t * DO + nck * NCK:t * DO + (nck + 1) * NCK],
                                start=(t == 0), stop=(t == n_ht - 1))
                        ev = oev.tile([128, NCK], F32, name=f"oe{qb}{stl}{nck}",
                                      tag="ev")
                        if nck % 2 == 0:
                            nc.vector.tensor_copy(ev[:], ps[:])
                        else:
                            nc.scalar.copy(ev[:], ps[:])
                        nc.sync.dma_start(out[st * 128:(st + 1) * 128,
                                              nck * NCK:(nck + 1) * NCK], ev[:])
                    thunks.append(th)
            return thunks

        def att_block(qb, bg):
            # attention for query block qb; interleave background thunks
            # (next proj chunk + previous outproj) into the kt stream.
            ktm = (qb + 1) * n_kq - 1
            n_iters = n_pairs * (ktm + 1)
            it = 0
            done = 0
            for p in range(n_pairs):
                he, ho = 2 * p, 2 * p + 1
                ctx_e = pctx.tile([VA, QB], F32, name=f"ce{qb}{p}", tag="ctx_e")
                ctx_o = pctx.tile([VA, QB], F32, name=f"co{qb}{p}", tag="ctx_o")
                q0 = qb * QB
                for kt in range(ktm + 1):
                    g = kt - qb * n_kq
                    qc0 = max(0, g) * 128
                    sct = psc.tile([128, 2 * QB], F32, name=f"s{qb}{p}{kt}",
                                   tag="sc")
                    sc3 = sct[:].rearrange("p (h c) -> p h c", c=QB)
                    nc.tensor.matmul(
                        sc3[:, 0, qc0:QB], kt_sb[p][0:DK, kt * 128:(kt + 1) * 128],
                        qt_sb[p][0:DK, q0 + qc0:q0 + QB], start=True, stop=True)
                    nc.tensor.matmul(
                        sc3[:, 1, qc0:QB], kt_sb[p][DK:2 * DK, kt * 128:(kt + 1) * 128],
                        qt_sb[p][DK:2 * DK, q0 + qc0:q0 + QB], start=True, stop=True)
                    ex = expool.tile([128, 2 * QB], BF16, name=f"e{qb}{p}{kt}",
                                     tag="ex")
                    ex3 = ex[:].rearrange("p (h c) -> p h c", c=QB)
                    nc.scalar.activation(ex3[:, :, qc0:QB], sc3[:, :, qc0:QB],
                                         AF.Exp, scale=scale)
                    # emit background PE work here: it executes while the
                    # scalar engine computes this iteration's exp, hiding the
                    # in-order PE stall at the AV matmul below.
                    it += 1
                    while done * n_iters < it * len(bg):
                        bg[done]()
                        done += 1
                    if g >= 0:
                        if fuse_mask:
                            nc.vector.tensor_mul(
                                ex3[:, :, qc0:qc0 + 128], ex3[:, :, qc0:qc0 + 128],
                                mask_sb[:].rearrange("p c -> p () c")
                                .broadcast_to([128, 2, 128]))
                        else:
                            for hh in range(2):
                                nc.vector.tensor_mul(
                                    ex3[:, hh, qc0:qc0 + 128],
                                    ex3[:, hh, qc0:qc0 + 128], mask_sb[:])
                    for (h, hh, cx) in ((he, 0, ctx_e), (ho, 1, ctx_o)):
                        vsl = v_aug[kt][:, h * VA:(h + 1) * VA]
                        if g >= 0:
                            nc.tensor.matmul(
                                cx[:, qc0:qc0 + 128], vsl,
                                ex3[:, hh, qc0:qc0 + 128],
                                start=(kt == 0), stop=True)
                            if qc0 + 128 < QB:
                                nc.tensor.matmul(
                                    cx[:, qc0 + 128:QB], vsl,
                                    ex3[:, hh, qc0 + 128:QB],
                                    start=(kt == 0), stop=False)
                        else:
                            nc.tensor.matmul(
                                cx[:], vsl, ex3[:, hh, 0:QB],
                                start=(kt == 0), stop=False)
                evac_and_normalize(he, qb, ctx_e)
                evac_and_normalize(ho, qb, ctx_o)
            while done < len(bg):
                bg[done]()
                done += 1

        def interleave(a, b):
            # spread b evenly through a
            res = list(a)
            for i, x in enumerate(b):
                res.insert((i + 1) * (len(a) + len(b)) // (len(b) + 1), x)
            return res

        # background-work assignment balances the PE's spare capacity against
        # each attention block's scalar-engine (exp) pacing: the last block
        # gets the two ripest outproj blocks, its projection having been
        # pulled one block earlier.
        for th in proj_thunks(0):
            th()
        for sc in range(n_sc):
            if sc + 2 <= n_sc - 1:
                dma_x(sc + 2)
            bg = proj_thunks(sc + 1) if sc + 1 < n_sc else []
            if sc > 0:
                bg = interleave(bg, outproj_thunks(sc - 1))
            att_block(sc, bg)
        for th in outproj_thunks(n_qb - 1):
            th()
        ctx.close()

    if split:
        split_multiwaits(nc)
    return nc


def core_inputs(queries, keys, values, Wq, bq, Wk, bk, Wv, bv, Wo, core, n_cores=8,
                HPC=None):
    """Host-side shard prep for one core. core -> (batch, head-group)."""
    import ml_dtypes
    bf16 = ml_dtypes.bfloat16
    B = queries.shape[0]
    H = Wq.shape[0]
    groups = n_cores // B
    b, hg = core // groups, core % groups
    if HPC is None:
        HPC = H // groups
    h0 = hg * HPC
    DK = Wq.shape[2]

    D = Wq.shape[1]
    HD = HPC * DK
    S = queries.shape[1]
    QB = 512
    n_dt, n_sc, n_ht = D // 128, S // QB, HD // 128

    def wsel(W):
        # [H, D, dk] -> [D, HPC*dk] head-major -> pre-tiled [128, n_dt*HD]
        w = W[h0:h0 + HPC].transpose(1, 0, 2).reshape(D, HD)
        return np.ascontiguousarray(
            w.reshape(n_dt, 128, HD).transpose(1, 0, 2).reshape(128, n_dt * HD)
        ).astype(bf16)

    def xsel(x):
        # [S, D] -> xT [D, S] -> pre-tiled [n_sc, 128, n_dt*QB]
        xT = x.T.reshape(n_dt, 128, n_sc, QB)
        return np.ascontiguousarray(
            xT.transpose(2, 1, 0, 3).reshape(n_sc, 128, n_dt * QB)).astype(bf16)

    def bpairs(bias):
        # [H, dk] -> [2*dk, HPC//2]
        bsel = bias[h0:h0 + HPC].reshape(HPC // 2, 2 * DK)
        return np.ascontiguousarray(bsel.T)

    wo_sel = Wo[h0 * DK:(h0 + HPC) * DK, :]
    DO = wo_sel.shape[1]
    wo_tiled = np.ascontiguousarray(
        wo_sel.reshape(n_ht, 128, DO).transpose(1, 0, 2).reshape(128, n_ht * DO))

    r = np.arange(128)
    mask = (r[None, :] >= r[:, None]).astype(np.float32)  # col >= row
    return {
        "mask_in": mask.astype(bf16),
        "vones": np.ones((128, HPC), bf16),
        "xqT": xsel(queries[b]),
        "xkT": xsel(keys[b]),
        "xvT": xsel(values[b]),
        "wq": wsel(Wq), "wk": wsel(Wk), "wv": wsel(Wv),
        "bqp": bpairs(bq), "bkp": bpairs(bk),
        "wo": wo_tiled,
    }


def assemble(results, B, n_cores, bias_total):
    """Sum head-group partials per batch and add the host-side bias."""
    groups = n_cores // B
    outs = []
    for b in range(B):
        acc = results[b * groups]["out"].astype(np.float64)
        for g in range(1, groups):
            acc = acc + results[b * groups + g]["out"]
        outs.append(acc + bias_total)
    return np.stack(outs).astype(np.float32)


# ---------------------------------------------------------------------------
# Harness entry point: full (unsharded) inputs -> full output.
# Shards batch (4) x head-halves (2) across the 8 NeuronCores, runs the Bass
# kernel via run_bass_kernel_spmd, then sums head-half partials per batch on
# the host (+ bias fold: out += bo + bv @ Wo, exact because attention rows
# sum to 1 after normalization).
# ---------------------------------------------------------------------------
_CACHE = {}


def kernel(**inputs):
    from concourse.bass_utils import run_bass_kernel_spmd

    queries = np.asarray(inputs["queries"], np.float32)
    keys = np.asarray(inputs["keys"], np.float32)
    values = np.asarray(inputs["values"], np.float32)
    Wq = np.asarray(inputs["Wq"], np.float32)
    bq = np.asarray(inputs["bq"], np.float32)
    Wk = np.asarray(inputs["Wk"], np.float32)
    bk = np.asarray(inputs["bk"], np.float32)
    Wv = np.asarray(inputs["Wv"], np.float32)
    bv = np.asarray(inputs["bv"], np.float32)
    Wo = np.asarray(inputs["Wo"], np.float32)
    bo = np.asarray(inputs["bo"], np.float32)

    B = queries.shape[0]
    n_cores = 8
    if "nc" not in _CACHE:
        _CACHE["nc"] = build()
    nc = _CACHE["nc"]
    in_maps = [core_inputs(queries, keys, values, Wq, bq, Wk, bk, Wv, bv, Wo,
                           core=c, n_cores=n_cores) for c in range(n_cores)]
    res = run_bass_kernel_spmd(nc, in_maps, list(range(n_cores)))
    bias_total = bo + bv.reshape(-1) @ Wo
    return assemble(res.results, B, n_cores, bias_total)


# revision 68
# speedup vs baseline: 1.0873x; 1.0668x over previous
"""Multi-head causal attention Bass/Tile kernel for TRN2.

Per-core program (SPMD across 8 cores): each core handles one batch b and
half the heads (HPC=8). Inputs arrive pre-transposed/sliced from the host
in bf16:
  xqT, xkT, xvT : [D, S]   (activations, transposed, bf16)
  wq, wk, wv    : [D, HPC*DK] bf16 (per-core head slice, head-major columns)
  bqp, bkp      : [2*DK, HPC//2]  (bias per head-pair column)
  wo            : [HPC*DK, DO] f32 (slice of Wo rows for these heads)
Output: out [S, DO] = normalized-attention context @ wo  (no bo; host adds
bo + bv@Wo and sums the two head-half partials).

Schedule: chunk-interleaved pipeline. The sequence is processed in 4 column
chunks of 512; for chunk sc we project Q/K/V columns [sc*512,(sc+1)*512) and
then immediately run attention for query block qb=sc (which only needs
K/V/Q chunks <= sc). The exp (scalar engine) work of attention block qb
overlaps the projection matmuls of chunk qb+1 that sit behind it in the PE
queue. The finished context block's output projection is interleaved into
the next attention block's stream.

Causal structure is exploited at 128-column granularity: for the diagonal
key tile g the score matmul and exp only cover query columns >= g*128, and
the mask multiply touches a single 128x128 tile.

Softmax denominators come from a ones-column augmented V (65th row of the
AV matmul output); reciprocals are computed by DMA-repacking the denominator
row across partitions (DVE reciprocal is free-dim serial).
"""

from contextlib import ExitStack

import numpy as np

import concourse.bass as bass
import concourse.mybir as mybir
import concourse.tile as tile
from concourse import library_config

F32 = mybir.dt.float32
F32R = mybir.dt.float32r
BF16 = mybir.dt.bfloat16
AF = mybir.ActivationFunctionType


def split_multiwaits(nc):
    """This walrus build accepts at most one sync-wait per instruction;
    hoist extra waits onto NOPs placed just before the instruction."""
    n_split = 0
    for fn in nc.m.functions:
        for blk in fn.blocks:
            insts = list(blk.instructions)
            out = []
            for inst in insts:
                si = inst.sync_info
                if si is not None and si.on_wait is not None and len(si.on_wait) > 1:
                    waits = list(si.on_wait)
                    for j, w in enumerate(waits[:-1]):
                        nop = mybir.InstNoOp(name=f"{inst.name}-sw{j}", ins=[], outs=[])
                        nop.engine = inst.engine
                        nop.sync_info = mybir.SyncInfo(on_wait=[w], on_update=[])
                        out.append(nop)
                    inst.sync_info = mybir.SyncInfo(
                        on_wait=[waits[-1]], on_update=list(si.on_update or [])
                    )
                    n_split += 1
                out.append(inst)
            if len(out) != len(insts):
                blk.instructions.clear()
                blk.instructions.extend(out)
    return n_split


def build(S=2048, D=1024, HPC=8, DK=64, DO=1024, QB=512, scale=0.125,
          split=True, fuse_mask=True):
    """Build the per-core Bass module. Returns nc."""
    assert S % QB == 0 and D % 128 == 0 and QB % 128 == 0
    n_dt = D // 128          # D tiles (contraction)
    n_sc = S // QB           # proj col chunks == q blocks
    n_qb = S // QB
    n_st = S // 128          # sequence tiles of 128 (key tiles)
    n_kq = QB // 128         # key tiles per q block
    n_pairs = HPC // 2
    HD = HPC * DK            # local head-concat dim
    n_ht = HD // 128         # ctx_stack tiles
    VA = DK + 1              # V augmented with ones column
    FQ = QB // 128
    NCK = 512                # outproj column chunk (f32r moving max 512)

    nc = bass.Bass("TRN2", target_bir_lowering=False, debug=False)

    # activations/weights arrive pre-tiled from the host so each DMA is one
    # contiguous run per partition: x [n_sc, 128, n_dt*QB], w [128, n_dt*HD]
    xqT = nc.dram_tensor("xqT", [n_sc, 128, n_dt * QB], BF16,
                         kind="ExternalInput").ap()
    xkT = nc.dram_tensor("xkT", [n_sc, 128, n_dt * QB], BF16,
                         kind="ExternalInput").ap()
    xvT = nc.dram_tensor("xvT", [n_sc, 128, n_dt * QB], BF16,
                         kind="ExternalInput").ap()
    wq = nc.dram_tensor("wq", [128, n_dt * HD], BF16, kind="ExternalInput").ap()
    wk = nc.dram_tensor("wk", [128, n_dt * HD], BF16, kind="ExternalInput").ap()
    wv = nc.dram_tensor("wv", [128, n_dt * HD], BF16, kind="ExternalInput").ap()
    bqp = nc.dram_tensor("bqp", [2 * DK, n_pairs], F32, kind="ExternalInput").ap()
    bkp = nc.dram_tensor("bkp", [2 * DK, n_pairs], F32, kind="ExternalInput").ap()
    wo = nc.dram_tensor("wo", [128, n_ht * DO], F32R, kind="ExternalInput").ap()
    mask_in = nc.dram_tensor("mask_in", [128, 128], BF16, kind="ExternalInput").ap()
    vones = nc.dram_tensor("vones", [128, HPC], BF16, kind="ExternalInput").ap()
    out = nc.dram_tensor("out", [S, DO], F32, kind="ExternalOutput").ap()

    with tile.TileContext(nc) as tc:
        ctx = ExitStack()
        # ---- persistent pools ----
        qk_pool = ctx.enter_context(tc.tile_pool(name="qk", bufs=1))
        va_pool = ctx.enter_context(tc.tile_pool(name="va", bufs=1))
        cs_pool = ctx.enter_context(tc.tile_pool(name="cs", bufs=1))
        w_pool = ctx.enter_context(tc.tile_pool(name="w", bufs=1))
        x_pool = ctx.enter_context(tc.tile_pool(name="x", bufs=1))
        small_pool = ctx.enter_context(tc.tile_pool(name="small", bufs=1))
        expool = ctx.enter_context(tc.tile_pool(name="ex", bufs=4))
        rcpool = ctx.enter_context(tc.tile_pool(name="rc", bufs=4))
        dscrpool = ctx.enter_context(tc.tile_pool(name="dscr", bufs=8, space="DRAM"))
        oev = ctx.enter_context(tc.tile_pool(name="oev", bufs=4))
        # PSUM: sc 2 bufs x 2 banks + ctx 2 banks + shared proj/outproj 2 banks
        psc = ctx.enter_context(tc.tile_pool(name="psc", bufs=2, space="PSUM"))
        pctx = ctx.enter_context(tc.tile_pool(name="pctx", bufs=1, space="PSUM"))
        pmix = ctx.enter_context(tc.tile_pool(name="pmix", bufs=2, space="PSUM"))

        qt_sb = [qk_pool.tile([2 * DK, S], BF16, name=f"qt{p}", tag=f"qt{p}")
                 for p in range(n_pairs)]
        kt_sb = [qk_pool.tile([2 * DK, S], BF16, name=f"kt{p}", tag=f"kt{p}")
                 for p in range(n_pairs)]
        v_aug = [va_pool.tile([128, HPC * VA], BF16, name=f"va{t}", tag=f"va{t}")
                 for t in range(n_st)]
        ctx_stack = [cs_pool.tile([128, S], F32R, name=f"cs{t}", tag=f"cs{t}")
                     for t in range(n_ht)]

        # ---- weights: one big DMA each (d-major free dim) ----
        wq_sb = w_pool.tile([128, n_dt * HD], BF16, tag="wq")
        wk_sb = w_pool.tile([128, n_dt * HD], BF16, tag="wk")
        wv_sb = w_pool.tile([128, n_dt * HD], BF16, tag="wv")
        wo_sb = w_pool.tile([128, n_ht * DO], F32R, tag="wo")
        x_sb = {}
        for t in ("q", "k", "v"):
            for sc in range(n_sc):
                x_sb[(t, sc)] = x_pool.tile([128, n_dt * QB], BF16,
                                            name=f"x{t}{sc}", tag=f"x{t}", bufs=2)

        bq_sb = small_pool.tile([2 * DK, n_pairs], F32, tag="bq")
        bk_sb = small_pool.tile([2 * DK, n_pairs], F32, tag="bk")
        mask_sb = small_pool.tile([128, 128], BF16, tag="mask")
        vones_sb = small_pool.tile([128, HPC], BF16, tag="vones")

        # critical-path-first DMA order, split across the two HWDGE queues
        # (sync + scalar) so the first Q-proj matmul unblocks in ~3us.
        def dma_xt(eng, t, xT, sc):
            eng.dma_start(x_sb[(t, sc)][:], xT[sc])

        def dma_x(sc):
            for (t, xT) in (("q", xqT), ("k", xkT), ("v", xvT)):
                dma_xt(nc.sync, t, xT, sc)

        nc.scalar.dma_start(wq_sb[:], wq[:])
        dma_xt(nc.sync, "q", xqT, 0)
        nc.sync.dma_start(wk_sb[:], wk[:])
        dma_xt(nc.scalar, "k", xkT, 0)
        nc.scalar.dma_start(wv_sb[:], wv[:])
        dma_xt(nc.sync, "v", xvT, 0)
        nc.sync.dma_start(bq_sb[:], bqp[:])
        nc.sync.dma_start(bk_sb[:], bkp[:])
        nc.sync.dma_start(mask_sb[:], mask_in[:])
        nc.sync.dma_start(vones_sb[:], vones[:])
        nc.scalar.dma_start(wo_sb[:], wo[:])
        if n_sc > 1:
            dma_xt(nc.sync, "q", xqT, 1)
            dma_xt(nc.scalar, "k", xkT, 1)
            dma_xt(nc.sync, "v", xvT, 1)

        def proj_thunks(sc):
            # Q and K projections for columns [sc*QB, (sc+1)*QB), as a list
            # of closures (one per pair / v-subtile; ~3.4us PE each) so they
            # can be interleaved into the previous attention block's stream.
            thunks = []
            for (w_all, b_sb, dst, t) in ((wq_sb, bq_sb, qt_sb, "q"),
                                          (wk_sb, bk_sb, kt_sb, "k")):
                xt = x_sb[(t, sc)]
                for p in range(n_pairs):
                    def th(w_all=w_all, b_sb=b_sb, dst=dst, t=t, xt=xt, p=p):
                        ps = pmix.tile([128, QB], F32, name=f"pj{t}{sc}{p}",
                                       tag="mix")
                        for d in range(n_dt):
                            nc.tensor.matmul(
                                ps[:],
                                w_all[:, d * HD + p * 128:d * HD + (p + 1) * 128],
                                xt[:, d * QB:(d + 1) * QB],
                                start=(d == 0), stop=(d == n_dt - 1))
                        nc.vector.tensor_scalar_add(
                            dst[p][:, sc * QB:(sc + 1) * QB], ps[:],
                            b_sb[:, p:p + 1])
                    thunks.append(th)
            xt = x_sb[("v", sc)]
            for stl in range(n_kq):
                def th(xt=xt, stl=stl):
                    st = sc * n_kq + stl
                    ps = pmix.tile([128, HD], F32, name=f"pjv{sc}{stl}", tag="mix")
                    for d in range(n_dt):
                        nc.tensor.matmul(
                            ps[:], xt[:, d * QB + stl * 128:d * QB + (stl + 1) * 128],
                            wv_sb[:, d * HD:(d + 1) * HD],
                            start=(d == 0), stop=(d == n_dt - 1))
                    va3 = v_aug[st][:].rearrange("p (h c) -> p h c", c=VA)
                    nc.scalar.copy(va3[:, :, 0:DK],
                                   ps[:].rearrange("p (h c) -> p h c", c=DK))
                    nc.vector.tensor_copy(va3[:, :, DK:VA],
                                          vones_sb[:].rearrange("p h -> p h ()"))
                thunks.append(th)
            return thunks

        def evac_and_normalize(h, qb, ctx_ps):
            # evacuate ctx (unnormalized) into ctx_stack + denom row, then
            # normalize in place once the PE-free recip chain lands. All hops
            # ride the sync HWDGE queue: triggers on the scalar queue would
            # block the ACT FIFO behind their data dependencies and stall exps.
            eng = nc.sync
            t, row0 = h // 2, (h % 2) * DK
            cs_slice = ctx_stack[t][row0:row0 + DK, qb * QB:(qb + 1) * QB]
            dn = rcpool.tile([1, QB], F32, name=f"dn{h}_{qb}", tag="dn")
            nc.vector.tensor_copy(dn[:], ctx_ps[DK:DK + 1, :])
            nc.vector.tensor_copy(cs_slice, ctx_ps[0:DK, :])
            ds1 = dscrpool.tile([1, QB], F32, name=f"ds1_{h}_{qb}", tag="ds1")
            eng.dma_start(ds1[:], dn[:])
            dnp = rcpool.tile([128, FQ], F32, name=f"dnp{h}_{qb}", tag="dnp")
            eng.dma_start(dnp[:], ds1[0, :].rearrange("(p f) -> p f", f=FQ))
            rcp = rcpool.tile([128, FQ], F32R, name=f"rcp{h}_{qb}", tag="rcp")
            with nc.allow_low_precision(reason="denom recip"):
                nc.vector.reciprocal(rcp[:], dnp[:])
            ds2 = dscrpool.tile([1, QB], F32R, name=f"ds2_{h}_{qb}", tag="ds2")
            eng.dma_start(ds2[0, :].rearrange("(p f) -> p f", f=FQ), rcp[:])
            bc_sb = rcpool.tile([128, QB], F32R, name=f"bc{h}_{qb}", tag="bc")
            eng.dma_start(bc_sb[row0:row0 + DK, :],
                          ds2[:].broadcast_to([DK, QB]))
            nc.vector.tensor_mul(cs_slice, cs_slice, bc_sb[row0:row0 + DK, :])

        def outproj_thunks(qb):
            # project finished ctx_stack columns (q rows qb*QB..) through wo
            thunks = []
            for stl in range(n_kq):
                st = qb * n_kq + stl
                for nck in range(DO // NCK):
                    def th(st=st, stl=stl, nck=nck):
                        ps = pmix.tile([128, NCK], F32, name=f"po{qb}{stl}{nck}",
                                       tag="mix")
                        for t in range(n_ht):
                            nc.tensor.matmul(
                                ps[:], ctx_stack[t][:, st * 128:(st + 1) * 128],
                                wo_sb[:, t * DO + nck * NCK:t * DO + (nck + 1) * NCK],
                                start=(t == 0), stop=(t == n_ht - 1))
                        ev = oev.tile([128, NCK], F32, name=f"oe{qb}{stl}{nck}",
                                      tag="ev")
                        if nck % 2 == 0:
                            nc.vector.tensor_copy(ev[:], ps[:])
                        else:
                            nc.scalar.copy(ev[:], ps[:])
                        # out-writes ride the idle gpsimd SWDGE queue so the
                        # sync HWDGE queue stays clear for the latency-
                        # critical softmax-denominator chains.
                        nc.gpsimd.dma_start(out[st * 128:(st + 1) * 128,
                                                nck * NCK:(nck + 1) * NCK], ev[:])
                    thunks.append(th)
            return thunks

        def att_block(qb, bg):
            # attention for query block qb; interleave background thunks
            # (next proj chunk + previous outproj) into the kt stream.
            ktm = (qb + 1) * n_kq - 1
            n_iters = n_pairs * (ktm + 1)
            it = 0
            done = 0
            for p in range(n_pairs):
                he, ho = 2 * p, 2 * p + 1
                ctx_e = pctx.tile([VA, QB], F32, name=f"ce{qb}{p}", tag="ctx_e")
                ctx_o = pctx.tile([VA, QB], F32, name=f"co{qb}{p}", tag="ctx_o")
                q0 = qb * QB
                for kt in range(ktm + 1):
                    g = kt - qb * n_kq
                    qc0 = max(0, g) * 128
                    sct = psc.tile([128, 2 * QB], F32, name=f"s{qb}{p}{kt}",
                                   tag="sc")
                    sc3 = sct[:].rearrange("p (h c) -> p h c", c=QB)
                    nc.tensor.matmul(
                        sc3[:, 0, qc0:QB], kt_sb[p][0:DK, kt * 128:(kt + 1) * 128],
                        qt_sb[p][0:DK, q0 + qc0:q0 + QB], start=True, stop=True)
                    nc.tensor.matmul(
                        sc3[:, 1, qc0:QB], kt_sb[p][DK:2 * DK, kt * 128:(kt + 1) * 128],
                        qt_sb[p][DK:2 * DK, q0 + qc0:q0 + QB], start=True, stop=True)
                    ex = expool.tile([128, 2 * QB], BF16, name=f"e{qb}{p}{kt}",
                                     tag="ex")
                    ex3 = ex[:].rearrange("p (h c) -> p h c", c=QB)
                    nc.scalar.activation(ex3[:, :, qc0:QB], sc3[:, :, qc0:QB],
                                         AF.Exp, scale=scale)
                    # emit background PE work here: it executes while the
                    # scalar engine computes this iteration's exp, hiding the
                    # in-order PE stall at the AV matmul below.
                    it += 1
                    while done * n_iters < it * len(bg):
                        bg[done]()
                        done += 1
                    if g >= 0:
                        if fuse_mask:
                            nc.vector.tensor_mul(
                                ex3[:, :, qc0:qc0 + 128], ex3[:, :, qc0:qc0 + 128],
                                mask_sb[:].rearrange("p c -> p () c")
                                .broadcast_to([128, 2, 128]))
                        else:
                            for hh in range(2):
                                nc.vector.tensor_mul(
                                    ex3[:, hh, qc0:qc0 + 128],
                                    ex3[:, hh, qc0:qc0 + 128], mask_sb[:])
                    for (h, hh, cx) in ((he, 0, ctx_e), (ho, 1, ctx_o)):
                        vsl = v_aug[kt][:, h * VA:(h + 1) * VA]
                        if g >= 0:
                            nc.tensor.matmul(
                                cx[:, qc0:qc0 + 128], vsl,
                                ex3[:, hh, qc0:qc0 + 128],
                                start=(kt == 0), stop=True)
                            if qc0 + 128 < QB:
                                nc.tensor.matmul(
                                    cx[:, qc0 + 128:QB], vsl,
                                    ex3[:, hh, qc0 + 128:QB],
                                    start=(kt == 0), stop=False)
                        else:
                            nc.tensor.matmul(
                                cx[:], vsl, ex3[:, hh, 0:QB],
                                start=(kt == 0), stop=False)
                evac_and_normalize(he, qb, ctx_e)
                evac_and_normalize(ho, qb, ctx_o)
            while done < len(bg):
                bg[done]()
                done += 1

        def interleave(a, b):
            # spread b evenly through a
            res = list(a)
            for i, x in enumerate(b):
                res.insert((i + 1) * (len(a) + len(b)) // (len(b) + 1), x)
            return res

        # background-work assignment balances the PE's spare capacity against
        # each attention block's scalar-engine (exp) pacing: the last block
        # gets the two ripest outproj blocks, its projection having been
        # pulled one block earlier.
        # background-work placement is balanced against each block's scalar-
        # engine pacing: att(n_sc-2) keeps all its PE slack for the last
        # projection chunk; its outproj debt moves into the final attention
        # block, which otherwise runs out of PE filler while ACT grinds exps.
        for th in proj_thunks(0):
            th()
        for sc in range(n_sc):
            if sc + 2 <= n_sc - 1:
                dma_x(sc + 2)
            if sc + 1 < n_sc:
                bg = proj_thunks(sc + 1)
                if sc > 0 and sc + 1 != n_sc - 1:
                    bg = interleave(bg, outproj_thunks(sc - 1))
            else:
                bg = interleave(outproj_thunks(sc - 2), outproj_thunks(sc - 1))
            att_block(sc, bg)
        for th in outproj_thunks(n_qb - 1):
            th()
        ctx.close()

    if split:
        split_multiwaits(nc)
    return nc


def core_inputs(queries, keys, values, Wq, bq, Wk, bk, Wv, bv, Wo, core, n_cores=8,
                HPC=None):
    """Host-side shard prep for one core. core -> (batch, head-group)."""
    import ml_dtypes
    bf16 = ml_dtypes.bfloat16
    B = queries.shape[0]
    H = Wq.shape[0]
    groups = n_cores // B
    b, hg = core // groups, core % groups
    if HPC is None:
        HPC = H // groups
    h0 = hg * HPC
    DK = Wq.shape[2]

    D = Wq.shape[1]
    HD = HPC * DK
    S = queries.shape[1]
    QB = 512
    n_dt, n_sc, n_ht = D // 128, S // QB, HD // 128

    def wsel(W):
        # [H, D, dk] -> [D, HPC*dk] head-major -> pre-tiled [128, n_dt*HD]
        w = W[h0:h0 + HPC].transpose(1, 0, 2).reshape(D, HD)
        return np.ascontiguousarray(
            w.reshape(n_dt, 128, HD).transpose(1, 0, 2).reshape(128, n_dt * HD)
        ).astype(bf16)

    def xsel(x):
        # [S, D] -> xT [D, S] -> pre-tiled [n_sc, 128, n_dt*QB]
        xT = x.T.reshape(n_dt, 128, n_sc, QB)
        return np.ascontiguousarray(
            xT.transpose(2, 1, 0, 3).reshape(n_sc, 128, n_dt * QB)).astype(bf16)

    def bpairs(bias):
        # [H, dk] -> [2*dk, HPC//2]
        bsel = bias[h0:h0 + HPC].reshape(HPC // 2, 2 * DK)
        return np.ascontiguousarray(bsel.T)

    wo_sel = Wo[h0 * DK:(h0 + HPC) * DK, :]
    DO = wo_sel.shape[1]
    wo_tiled = np.ascontiguousarray(
        wo_sel.reshape(n_ht, 128, DO).transpose(1, 0, 2).reshape(128, n_ht * DO))

    r = np.arange(128)
    mask = (r[None, :] >= r[:, None]).astype(np.float32)  # col >= row
    return {
        "mask_in": mask.astype(bf16),
        "vones": np.ones((128, HPC), bf16),
        "xqT": xsel(queries[b]),
        "xkT": xsel(keys[b]),
        "xvT": xsel(values[b]),
        "wq": wsel(Wq), "wk": wsel(Wk), "wv": wsel(Wv),
        "bqp": bpairs(bq), "bkp": bpairs(bk),
        "wo": wo_tiled,
    }


def assemble(results, B, n_cores, bias_total):
    """Sum head-group partials per batch and add the host-side bias."""
    groups = n_cores // B
    outs = []
    for b in range(B):
        acc = results[b * groups]["out"].astype(np.float64)
        for g in range(1, groups):
            acc = acc + results[b * groups + g]["out"]
        outs.append(acc + bias_total)
    return np.stack(outs).astype(np.float32)


# ---------------------------------------------------------------------------
# Harness entry point: full (unsharded) inputs -> full output.
# Shards batch (4) x head-halves (2) across the 8 NeuronCores, runs the Bass
# kernel via run_bass_kernel_spmd, then sums head-half partials per batch on
# the host (+ bias fold: out += bo + bv @ Wo, exact because attention rows
# sum to 1 after normalization).
# ---------------------------------------------------------------------------
_CACHE = {}


def kernel(**inputs):
    from concourse.bass_utils import run_bass_kernel_spmd

    queries = np.asarray(inputs["queries"], np.float32)
    keys = np.asarray(inputs["keys"], np.float32)
    values = np.asarray(inputs["values"], np.float32)
    Wq = np.asarray(inputs["Wq"], np.float32)
    bq = np.asarray(inputs["bq"], np.float32)
    Wk = np.asarray(inputs["Wk"], np.float32)
    bk = np.asarray(inputs["bk"], np.float32)
    Wv = np.asarray(inputs["Wv"], np.float32)
    bv = np.asarray(inputs["bv"], np.float32)
    Wo = np.asarray(inputs["Wo"], np.float32)
    bo = np.asarray(inputs["bo"], np.float32)

    B = queries.shape[0]
    n_cores = 8
    if "nc" not in _CACHE:
        _CACHE["nc"] = build()
    nc = _CACHE["nc"]
    in_maps = [core_inputs(queries, keys, values, Wq, bq, Wk, bk, Wv, bv, Wo,
                           core=c, n_cores=n_cores) for c in range(n_cores)]
    res = run_bass_kernel_spmd(nc, in_maps, list(range(n_cores)))
    bias_total = bo + bv.reshape(-1) @ Wo
    return assemble(res.results, B, n_cores, bias_total)
